# revision 16
# baseline (speedup 1.0000x reference)
"""Trainium2 Bass kernel for nn_GCN2_12893491822964 (8-layer GCN2, N=50000,
E=800000, IN=128, HID=64, OUT=40) on 8 NeuronCores.

Strategy (node/dst sharding, SPMD single program):
  - Each core owns 6250 dst nodes, split into region A (windows 0-24) and
    region B (windows 25-49); regions are balanced by out-degree so each
    holds ~half the edge mass.  Within a region, dsts are packed into
    windows of 128 lanes by an LPT heuristic so every (window, src-region)
    holds <= 1024 edges -> exactly 8 slot-tiles.  The per-core
    dst->(window,lane) permutation is absorbed into the input/x0 layout,
    gather index tables and output unpermute.
  - The replicated h table is split into TWO region tables T_A/T_B
    [8*3200=25600, 64] f32 (int16-indexable).  Region A of the next layer
    is AllGathered MID-layer (fully hidden); region B at layer end, hidden
    behind the next layer's region-A gathers (emission reordered A-first).
  - Messages h[src] are fetched with GPSIMD dma_gather (256B rows); calls
    round-robin over 4 SWDGE queues so all 8 Q7 cores generate descriptors
    concurrently.
  - Segment-sum on the PE: per tile, a 0/1 selector S_t [128 slot, 128 dst]
    (bf16, built on-chip by one DVE is_equal over iota vs a per-slot
    dst-lane table) contracts gathered messages (scaled by the per-edge
    weight and cast to bf16 in one DVE pass) into a PSUM accumulator per
    window: psum[dst, f] += S_t^T @ (w*M)_t.
  - Dense part per window: u = psum + 0.1*x0; PE transpose; h = relu(u @
    Wt_l) with Wt_l = (1-b)I + b*W_l folded on host.  Entry (bf16) / exit
    matmuls run per window batch on PE.
"""
import numpy as np

N, E, IN_CH, HID, OUT_CH, L = 50000, 800000, 128, 64, 40, 8
ALPHA, THETA = 0.1, 0.5
C = 8
SHARD = N // C            # 6250
W = 50                    # windows of 128 lanes
WR = 25                   # windows per region
PS = W * 128              # 6400 padded shard
RS = WR * 128             # 3200 region rows per core
TROWS = C * RS            # 25600 rows per region table
TLW = 8                   # slot tiles per (window, src-region)
GW = 2                    # windows per gather group
NT = GW * TLW             # tiles per (group, half) chunk = 16
NG = W // GW              # 25 groups
TH = W * TLW              # 400 tiles per half
AG_A_GROUP = 12           # after this group, windows 0..25 done -> AG A


# ---------------------------------------------------------------- host prep

def _pack_region(ld, hd):
    """Assign region dsts to (window, lane): per (window, half) <= 1024."""
    n = len(ld)
    order = np.argsort(-(ld + hd), kind='stable')
    lo = np.zeros(WR)
    hi = np.zeros(WR)
    cnt = np.zeros(WR, np.int64)
    wdst = np.empty(n, np.int64)
    for d in order:
        ol = np.maximum(lo + ld[d] - 1024, 0)
        oh = np.maximum(hi + hd[d] - 1024, 0)
        score = (ol + oh) * 1e6 + (lo + hi) + 16.0 * cnt \
            + np.where(cnt >= 128, 1e12, 0)
        w = int(np.argmin(score))
        wdst[d] = w
        lo[w] += ld[d]
        hi[w] += hd[d]
        cnt[w] += 1
    # repair: move single dsts out of over-cap windows (aggregate slack is
    # ~3%, so a feasible move nearly always exists)
    for _ in range(2000):
        viol = np.where((lo > 1024) | (hi > 1024))[0]
        if len(viol) == 0:
            break
        w = int(viol[0])
        members = np.where(wdst == w)[0]
        members = members[np.argsort(-(ld[members] + hd[members]))]
        moved = False
        for d in members:
            fit = (cnt < 128) & (lo + ld[d] <= 1024) & (hi + hd[d] <= 1024)
            fit[w] = False
            if (lo[w] - ld[d] <= 1024) and (hi[w] - hd[d] <= 1024) \
                    and fit.any():
                w2 = int(np.argmin(np.where(fit, lo + hi, np.inf)))
                wdst[d] = w2
                lo[w] -= ld[d]; hi[w] -= hd[d]; cnt[w] -= 1
                lo[w2] += ld[d]; hi[w2] += hd[d]; cnt[w2] += 1
                moved = True
                break
        if not moved:
            # move the heaviest member anywhere with count room
            d = members[0]
            fit = cnt < 128
            fit[w] = False
            w2 = int(np.argmin(np.where(fit, (np.maximum(lo + ld[d] - 1024, 0)
                                              + np.maximum(hi + hd[d] - 1024, 0))
                                        * 1e6 + lo + hi, np.inf)))
            wdst[d] = w2
            lo[w] -= ld[d]; hi[w] -= hd[d]; cnt[w] -= 1
            lo[w2] += ld[d]; hi[w2] += hd[d]; cnt[w2] += 1
    assert lo.max() <= 1024 and hi.max() <= 1024 and cnt.max() <= 128, \
        (lo.max(), hi.max(), cnt.max())
    pos = np.zeros(n, np.int64)
    c2 = np.zeros(WR, np.int64)
    for d in range(n):
        pos[d] = c2[wdst[d]]
        c2[wdst[d]] += 1
    return wdst * 128 + pos


def _build_structures(edge_src, edge_dst, edge_weight):
    src = np.asarray(edge_src, np.int64)
    dst = np.asarray(edge_dst, np.int64)
    wgt = np.asarray(edge_weight, np.float32)

    # per-core region split balancing BOTH out-degree mass (bounds src-half
    # edge totals) and in-degree mass (bounds dst-window packing), size <= RS
    outdeg = np.bincount(src, minlength=N)
    indeg = np.bincount(dst, minlength=N)
    regB = np.zeros(N, bool)
    for c in range(C):
        od = outdeg[c * SHARD:(c + 1) * SHARD].astype(np.float64)
        idg = indeg[c * SHARD:(c + 1) * SHARD].astype(np.float64)
        order = np.argsort(-(od + idg), kind='stable')
        b = np.zeros(SHARD, bool)
        mo = np.zeros(2)
        mi = np.zeros(2)
        cnt2 = np.zeros(2, np.int64)
        for d in order:
            s0 = max(mo[0] + od[d], mi[0] + idg[d]) + (1e12 if cnt2[0] >= RS else 0)
            s1 = max(mo[1] + od[d], mi[1] + idg[d]) + (1e12 if cnt2[1] >= RS else 0)
            r = 0 if s0 <= s1 else 1
            b[d] = bool(r)
            mo[r] += od[d]
            mi[r] += idg[d]
            cnt2[r] += 1
        regB[c * SHARD:(c + 1) * SHARD] = b
        assert (~b).sum() <= RS and b.sum() <= RS
    e_half = regB[src].astype(np.int64)           # edge table half by src

    # per-core packing (needs per-dst degree split by src region)
    packed = np.zeros(N, np.int64)
    for c in range(C):
        mc = (dst // SHARD) == c
        ldst = dst[mc] % SHARD
        hs = e_half[mc]
        ld = np.bincount(ldst[hs == 0], minlength=SHARD)
        hd = np.bincount(ldst[hs == 1], minlength=SHARD)
        b = regB[c * SHARD:(c + 1) * SHARD]
        pk = np.empty(SHARD, np.int64)
        idxA = np.where(~b)[0]
        idxB = np.where(b)[0]
        pk[idxA] = _pack_region(ld[idxA], hd[idxA])
        pk[idxB] = RS + _pack_region(ld[idxB], hd[idxB])
        packed[c * SHARD:(c + 1) * SHARD] = pk

    # table row of a node within its region table: c*RS + (pos % RS)
    g_row = (src // SHARD) * RS + (packed[src] % RS)
    assert g_row.max() < TROWS <= 32768

    tables = []
    perms = []
    for c in range(C):
        mc = (dst // SHARD) == c
        pk = packed[c * SHARD:(c + 1) * SHARD]
        perms.append(pk)
        cs = g_row[mc]
        chh = e_half[mc]
        cd = pk[dst[mc] % SHARD]
        cw = wgt[mc] * np.float32(1.0 - ALPHA)
        order = np.lexsort((np.arange(len(cs)), cd, chh))
        cs, chh, cd, cw = cs[order], chh[order], cd[order], cw[order]
        core_t = []
        for h in (0, 1):
            sel = chh == h
            ps_, ss, ws_ = cd[sel], cs[sel], cw[sel]
            wsel = ps_ // 128
            cnt = np.bincount(wsel, minlength=W)
            assert cnt.max() <= TLW * 128
            starts = np.concatenate([[0], np.cumsum(cnt)[:-1]])
            r = np.arange(int(sel.sum())) - starts[wsel]
            tile_i = wsel * TLW + r // 128
            lane = r % 128
            idx = np.zeros((TH * 128,), np.int64)
            wp = np.zeros((128, TH), np.float32)
            dl = np.full((128, TH), -1.0, np.float32)
            idx[tile_i * 128 + lane] = ss
            wp[lane, tile_i] = ws_
            dl[lane, tile_i] = np.float32(ps_ % 128)
            core_t.append((idx.reshape(TH, 128), wp, dl))
        tables.append(core_t)
    return tables, perms


def _pack_idx(idx_t128):
    """[T,128] int ->  [128, T*8] int16 gather table (pos i=t*128+p)."""
    flat = np.asarray(idx_t128, np.int16).reshape(-1)     # i = t*128+p
    blk = flat.reshape(-1, 16).T                          # [16, T*8]
    return np.tile(blk, (8, 1)).copy()                    # [128, T*8]


# ---------------------------------------------------------------- bass build

def _build_nc():
    import concourse.bass as bass
    import concourse.bacc as bacc
    import concourse.tile as tile
    import concourse.mybir as mybir

    f32, i16, bf16 = mybir.dt.float32, mybir.dt.int16, mybir.dt.bfloat16
    Alu, Act = mybir.AluOpType, mybir.ActivationFunctionType

    nc = bacc.Bacc("TRN2", target_bir_lowering=False, debug=False,
                   num_devices=C, num_swdge_queues=4)

    xT_d = nc.dram_tensor("xT", [IN_CH, PS], bf16, kind="ExternalInput")
    idx_d = [nc.dram_tensor(f"idx{h}", [128, TH * 8], i16, kind="ExternalInput")
             for h in (0, 1)]
    wp_d = [nc.dram_tensor(f"wp{h}", [128, TH], f32, kind="ExternalInput")
            for h in (0, 1)]
    dl_d = [nc.dram_tensor(f"dl{h}", [128, TH], bf16, kind="ExternalInput")
            for h in (0, 1)]
    iota_d = nc.dram_tensor("iota", [128, 128], bf16, kind="ExternalInput")
    Win_d = nc.dram_tensor("Win", [IN_CH, HID], bf16, kind="ExternalInput")
    bin_d = nc.dram_tensor("bin", [HID, 1], f32, kind="ExternalInput")
    Wt_d = nc.dram_tensor("Wt", [HID, L * HID], f32, kind="ExternalInput")
    Wout_d = nc.dram_tensor("Wout", [HID, OUT_CH], f32, kind="ExternalInput")
    bout_d = nc.dram_tensor("bout", [OUT_CH, 1], f32, kind="ExternalInput")
    id64_d = nc.dram_tensor("id64", [64, 64], f32, kind="ExternalInput")
    id128_d = nc.dram_tensor("id128", [128, 128], f32, kind="ExternalInput")
    outT_d = nc.dram_tensor("outT", [OUT_CH, PS], f32, kind="ExternalOutput")

    with tile.TileContext(nc) as tc:
        with tc.tile_pool(name="const", bufs=1) as cp, \
             tc.tile_pool(name="msg", bufs=6) as m_pool, \
             tc.tile_pool(name="msgb", bufs=6) as mb_pool, \
             tc.tile_pool(name="sel", bufs=5) as s_pool, \
             tc.tile_pool(name="work", bufs=3) as wp_pool, \
             tc.tile_pool(name="psA", bufs=2, space="PSUM") as psA, \
             tc.tile_pool(name="psB", bufs=2, space="PSUM") as psB, \
             tc.tile_pool(name="psC", bufs=2, space="PSUM") as psC, \
             tc.tile_pool(name="dram", bufs=1, space="DRAM") as dram:

            # ---- persistent SBUF
            xT = cp.tile([IN_CH, PS], bf16)
            nc.sync.dma_start(xT[:], xT_d.ap())
            idx_sb = [cp.tile([128, TH * 8], i16, name=f"idxsb{h}") for h in (0, 1)]
            wp_sb = [cp.tile([128, TH], f32, name=f"wpsb{h}") for h in (0, 1)]
            dl_sb = [cp.tile([128, TH], bf16, name=f"dlsb{h}") for h in (0, 1)]
            for h in (0, 1):
                nc.sync.dma_start(idx_sb[h][:], idx_d[h].ap())
                nc.sync.dma_start(wp_sb[h][:], wp_d[h].ap())
                nc.sync.dma_start(dl_sb[h][:], dl_d[h].ap())
            iota = cp.tile([128, 128], bf16)
            nc.sync.dma_start(iota[:], iota_d.ap())
            Win = cp.tile([IN_CH, HID], bf16)
            nc.sync.dma_start(Win[:], Win_d.ap())
            b_in = cp.tile([HID, 1], f32)
            nc.sync.dma_start(b_in[:], bin_d.ap())
            Wt = cp.tile([HID, L * HID], f32)
            nc.sync.dma_start(Wt[:], Wt_d.ap())
            Wout = cp.tile([HID, OUT_CH], f32)
            nc.sync.dma_start(Wout[:], Wout_d.ap())
            b_out = cp.tile([OUT_CH, 1], f32)
            nc.sync.dma_start(b_out[:], bout_d.ap())
            id64 = cp.tile([64, 64], f32)
            nc.sync.dma_start(id64[:], id64_d.ap())
            id128 = cp.tile([128, 128], f32)
            nc.sync.dma_start(id128[:], id128_d.ap())

            x0s = cp.tile([128, W * HID], f32)       # 0.1*x0, node-major
            h_stage = cp.tile([128, W * HID], f32)   # new h, node-major
            outT_st = cp.tile([OUT_CH, PS], f32)

            h_sh = [[dram.tile([RS, HID], f32, name=f"h_sh{i}_{r}",
                               tag=f"hs{i}{r}") for r in (0, 1)]
                    for i in range(L)]
            h_full = [[dram.tile([TROWS, HID], f32, addr_space="Shared",
                                 name=f"h_full{i}_{r}", tag=f"hf{i}{r}")
                       for r in (0, 1)] for i in range(L)]
            h_stage_r = h_stage[:].rearrange("p (w f) -> p w f", f=HID)

            def writeback(i, w0, w1):
                # windows [w0, w1) of layer i's h -> region shard rows
                r = 0 if w0 < WR else 1
                assert (w1 <= WR) == (r == 0)
                hs_r = h_sh[i][r][:].rearrange("(w p) f -> p w f", p=128)
                nc.sync.dma_start(hs_r[:, w0 - r * WR:w1 - r * WR, :],
                                  h_stage_r[:, w0:w1, :])

            def allgather(i, r):
                nc.gpsimd.collective_compute(
                    "AllGather", Alu.bypass,
                    replica_groups=[list(range(C))],
                    ins=[h_sh[i][r].opt()], outs=[h_full[i][r].opt()])

            # ---- entry: h0 = relu(x @ Win + b), x0s = 0.1*h0 (4 windows/mm)
            for w0 in range(0, W, 4):
                nb = min(4, W - w0)
                pe = psA.tile([HID, 512], f32, name="pe", tag="psA")
                nc.tensor.matmul(pe[:, 0:nb * 128], lhsT=Win[:],
                                 rhs=xT[:, w0 * 128:(w0 + nb) * 128],
                                 start=True, stop=True)
                h0T = wp_pool.tile([HID, 512], f32, name="h0T")
                nc.scalar.activation(h0T[:, 0:nb * 128], pe[:, 0:nb * 128],
                                     Act.Relu, bias=b_in[:, 0:1])
                for k in range(nb):
                    w = w0 + k
                    pt = psB.tile([128, HID], f32, name="pt", tag="psB")
                    nc.tensor.transpose(pt[:], h0T[:, k * 128:(k + 1) * 128],
                                        id64[:])
                    nc.vector.tensor_copy(h_stage[:, w * HID:(w + 1) * HID],
                                          pt[:])
                    nc.vector.tensor_scalar_mul(
                        x0s[:, w * HID:(w + 1) * HID], pt[:], 0.1)
                if w0 + nb == 28:                 # windows 0..24 ready
                    writeback(0, 0, WR)
                    allgather(0, 0)
            writeback(0, WR, W)
            allgather(0, 1)

            qctr = 0
            # ---- layers
            for l in range(L):
                h_half = [h_full[l][0], h_full[l][1]]

                gathered = {}     # (gi, h) -> (mp, s_sb)

                def emit_gather(gi, h):
                    nonlocal qctr
                    t0 = gi * NT
                    s_sb = s_pool.tile([128, NT * 128], bf16,
                                       name=f"s{h}", tag=f"s{h}")
                    s3 = s_sb[:].rearrange("p (t j) -> p t j", j=128)
                    io_b = iota[:].unsqueeze(1).broadcast_to([128, NT, 128])
                    dl_b = dl_sb[h][:, t0:t0 + NT].unsqueeze(2) \
                        .broadcast_to([128, NT, 128])
                    nc.vector.tensor_tensor(s3, io_b, dl_b, Alu.is_equal)
                    m = m_pool.tile([128, NT * 64], f32,
                                    name=f"m{h}", tag=f"m{h}")
                    nc.gpsimd.dma_gather(
                        out_ap=m[:].rearrange("p (t f) -> p t f", f=64),
                        in_ap=h_half[h],
                        idxs_ap=idx_sb[h][:, t0 * 8:(t0 + NT) * 8],
                        num_idxs=NT * 128,
                        num_idxs_reg=NT * 128,
                        elem_size=HID,
                        single_packet=False,
                        queue_num=qctr % 4,
                    )
                    qctr += 1
                    wb = wp_sb[h][:, t0:t0 + NT].unsqueeze(2) \
                        .broadcast_to([128, NT, 64])
                    m3 = m[:].rearrange("p (t f) -> p t f", f=64)
                    mp = mb_pool.tile([128, NT * 64], bf16,
                                      name=f"mp{h}", tag=f"mp{h}")
                    mp3 = mp[:].rearrange("p (t f) -> p t f", f=64)
                    nc.vector.tensor_tensor(mp3, m3, wb, Alu.mult)
                    gathered[(gi, h)] = (mp, s_sb)

                # ---- phase 1: all region-A gathers; accumulate the A-half
                # of each window's segment-sum into h_stage (scratch):
                # h_stage[w] = psumA + 0.1*x0.  The long A-burst keeps the
                # Pool queue busy while AG-B (doorbelled at the end of the
                # previous layer) completes.
                for gi in range(NG):
                    emit_gather(gi, 0)
                    mp, s_sb = gathered[(gi, 0)]
                    for k in range(GW):
                        w = gi * GW + k
                        ps = psA.tile([128, HID], f32, name="agg", tag="psA")
                        for t in range(TLW):
                            to = k * TLW + t
                            nc.tensor.matmul(
                                ps[:],
                                lhsT=s_sb[:, to * 128:(to + 1) * 128],
                                rhs=mp[:, to * 64:(to + 1) * 64],
                                start=(t == 0), stop=(t == TLW - 1))
                        nc.vector.tensor_tensor(
                            h_stage[:, w * HID:(w + 1) * HID], ps[:],
                            x0s[:, w * HID:(w + 1) * HID], Alu.add)
                    del gathered[(gi, 0)]

                # ---- phase 2: region-B gathers; finalize each window
                wb_mark = 0
                for gi in range(NG):
                    emit_gather(gi, 1)
                    mp, s_sb = gathered[(gi, 1)]
                    for k in range(GW):
                        w = gi * GW + k
                        ps = psA.tile([128, HID], f32, name="agg", tag="psA")
                        for t in range(TLW):
                            to = k * TLW + t
                            nc.tensor.matmul(
                                ps[:],
                                lhsT=s_sb[:, to * 128:(to + 1) * 128],
                                rhs=mp[:, to * 64:(to + 1) * 64],
                                start=(t == 0), stop=(t == TLW - 1))
                        u = wp_pool.tile([128, HID], f32, name="u")
                        nc.vector.tensor_tensor(
                            u[:], ps[:], h_stage[:, w * HID:(w + 1) * HID],
                            Alu.add)
                        pt = psB.tile([HID, 128], f32, name="ptu", tag="psB")
                        nc.tensor.transpose(pt[:], u[:], id128[:])
                        uT = wp_pool.tile([HID, 128], f32, name="uT")
                        nc.vector.tensor_copy(uT[:], pt[:])
                        if l < L - 1:
                            pd = psC.tile([128, HID], f32, name="pd", tag="psC")
                            nc.tensor.matmul(
                                pd[:], lhsT=uT[:],
                                rhs=Wt[:, l * HID:(l + 1) * HID],
                                start=True, stop=True)
                            nc.scalar.activation(
                                h_stage[:, w * HID:(w + 1) * HID], pd[:],
                                Act.Relu)
                        else:
                            pd = psC.tile([HID, 128], f32, name="pdT", tag="psC")
                            nc.tensor.matmul(
                                pd[:], lhsT=Wt[:, l * HID:(l + 1) * HID],
                                rhs=uT[:], start=True, stop=True)
                            hT = wp_pool.tile([HID, 128], f32, name="hT")
                            nc.scalar.activation(hT[:], pd[:], Act.Relu)
                            px = psA.tile([OUT_CH, 128], f32, name="px",
                                          tag="psA")
                            nc.tensor.matmul(px[:], lhsT=Wout[:], rhs=hT[:],
                                             start=True, stop=True)
                            nc.vector.tensor_scalar_add(
                                outT_st[:, w * 128:(w + 1) * 128], px[:],
                                b_out[:, 0:1])
                    del gathered[(gi, 1)]
                    if l < L - 1:
                        done_w = (gi + 1) * GW
                        if gi == AG_A_GROUP:
                            writeback(l + 1, wb_mark, WR)
                            allgather(l + 1, 0)
                            wb_mark = WR
                        elif done_w - wb_mark >= 10 and done_w <= WR:
                            writeback(l + 1, wb_mark, done_w)
                            wb_mark = done_w
                        elif wb_mark >= WR and done_w - wb_mark >= 10 \
                                and done_w < W:
                            writeback(l + 1, wb_mark, done_w)
                            wb_mark = done_w
                if l < L - 1:
                    if wb_mark < W:
                        writeback(l + 1, wb_mark, W)
                    allgather(l + 1, 1)

            nc.sync.dma_start(outT_d.ap(), outT_st[:])

    nc.compile()
    return nc


# ---------------------------------------------------------------- entry point

_CACHE = {}


def kernel(x, edge_src, edge_dst, edge_weight, W_in, b_in, W_convs, W_out,
           b_out):
    import ml_dtypes
    from concourse.bass_utils import run_bass_kernel_spmd

    x = np.asarray(x, np.float32)
    W_in = np.asarray(W_in, np.float32)
    b_in_a = np.asarray(b_in, np.float32)
    W_convs = np.asarray(W_convs, np.float32)
    W_out_a = np.asarray(W_out, np.float32)
    b_out_a = np.asarray(b_out, np.float32)

    tables, perms = _build_structures(edge_src, edge_dst, edge_weight)

    if 'nc' not in _CACHE:
        _CACHE['nc'] = _build_nc()
    nc = _CACHE['nc']

    # folded per-layer weights
    Wt = np.zeros((HID, L * HID), np.float32)
    for l in range(L):
        beta = np.float32(np.log(THETA / (l + 1) + 1.0))
        Wt[:, l * HID:(l + 1) * HID] = \
            (1 - beta) * np.eye(HID, dtype=np.float32) + beta * W_convs[l]

    in_maps = []
    for c in range(C):
        xs = np.zeros((PS, IN_CH), np.float32)
        xs[perms[c]] = x[c * SHARD:(c + 1) * SHARD]
        in_maps.append({
            "xT": np.ascontiguousarray(xs.T).astype(ml_dtypes.bfloat16),
            "idx0": _pack_idx(tables[c][0][0]),
            "idx1": _pack_idx(tables[c][1][0]),
            "wp0": tables[c][0][1],
            "wp1": tables[c][1][1],
            "dl0": tables[c][0][2].astype(ml_dtypes.bfloat16),
            "dl1": tables[c][1][2].astype(ml_dtypes.bfloat16),
            "iota": np.tile(np.arange(128, dtype=np.float32),
                            (128, 1)).astype(ml_dtypes.bfloat16),
            "Win": W_in.astype(ml_dtypes.bfloat16),
            "bin": b_in_a.reshape(HID, 1),
            "Wt": Wt,
            "Wout": W_out_a,
            "bout": b_out_a.reshape(OUT_CH, 1),
            "id64": np.eye(64, dtype=np.float32),
            "id128": np.eye(128, dtype=np.float32),
        })

    res = run_bass_kernel_spmd(nc, in_maps, core_ids=list(range(C)))
    globals()['_LAST_RESULTS'] = res

    out = np.zeros((N, OUT_CH), np.float32)
    for c in range(C):
        oT = res.results[c]["outT"]          # [40, PS]
        out[c * SHARD:(c + 1) * SHARD] = oT[:, perms[c]].T
    return out


# revision 17
# speedup vs baseline: 1.0053x; 1.0053x over previous
"""Trainium2 Bass kernel for nn_GCN2_12893491822964 (8-layer GCN2, N=50000,
E=800000, IN=128, HID=64, OUT=40) on 8 NeuronCores.

Strategy (node/dst sharding, SPMD single program):
  - Each core owns 6250 dst nodes, split into region A (windows 0-24) and
    region B (windows 25-49); regions are balanced by out-degree so each
    holds ~half the edge mass.  Within a region, dsts are packed into
    windows of 128 lanes by an LPT heuristic so every (window, src-region)
    holds <= 1024 edges -> exactly 8 slot-tiles.  The per-core
    dst->(window,lane) permutation is absorbed into the input/x0 layout,
    gather index tables and output unpermute.
  - The replicated h table is split into TWO region tables T_A/T_B
    [8*3200=25600, 64] f32 (int16-indexable).  Region A of the next layer
    is AllGathered MID-layer (fully hidden); region B at layer end, hidden
    behind the next layer's region-A gathers (emission reordered A-first).
  - Messages h[src] are fetched with GPSIMD dma_gather (256B rows); calls
    round-robin over 4 SWDGE queues so all 8 Q7 cores generate descriptors
    concurrently.
  - Segment-sum on the PE: per tile, a 0/1 selector S_t [128 slot, 128 dst]
    (bf16, built on-chip by one DVE is_equal over iota vs a per-slot
    dst-lane table) contracts gathered messages (scaled by the per-edge
    weight and cast to bf16 in one DVE pass) into a PSUM accumulator per
    window: psum[dst, f] += S_t^T @ (w*M)_t.
  - Dense part per window: u = psum + 0.1*x0; PE transpose; h = relu(u @
    Wt_l) with Wt_l = (1-b)I + b*W_l folded on host.  Entry (bf16) / exit
    matmuls run per window batch on PE.
"""
import numpy as np

N, E, IN_CH, HID, OUT_CH, L = 50000, 800000, 128, 64, 40, 8
ALPHA, THETA = 0.1, 0.5
C = 8
SHARD = N // C            # 6250
W = 50                    # windows of 128 lanes
WR = 25                   # windows per region
PS = W * 128              # 6400 padded shard
RS = WR * 128             # 3200 region rows per core
TROWS = C * RS            # 25600 rows per region table
TLW = 8                   # slot tiles per (window, src-region)
GW = 2                    # windows per gather group
NT = GW * TLW             # tiles per (group, half) chunk = 16
NG = W // GW              # 25 groups
TH = W * TLW              # 400 tiles per half
AG_A_GROUP = 12           # after this group, windows 0..25 done -> AG A

# SWDGE queue pattern: weight calls toward the queues measured cheaper on HW
_QW = (1 / 5.18, 1 / 6.33, 1 / 4.82, 1 / 7.14)
QPAT = []
_load = [0.0] * 4
for _ in range(50):
    _q = min(range(4), key=lambda i: (_load[i] + 1) / _QW[i])
    QPAT.append(_q)
    _load[_q] += 1.0


# ---------------------------------------------------------------- host prep

def _pack_region(ld, hd):
    """Assign region dsts to (window, lane): per (window, half) <= 1024."""
    n = len(ld)
    order = np.argsort(-(ld + hd), kind='stable')
    lo = np.zeros(WR)
    hi = np.zeros(WR)
    cnt = np.zeros(WR, np.int64)
    wdst = np.empty(n, np.int64)
    for d in order:
        ol = np.maximum(lo + ld[d] - 1024, 0)
        oh = np.maximum(hi + hd[d] - 1024, 0)
        score = (ol + oh) * 1e6 + (lo + hi) + 16.0 * cnt \
            + np.where(cnt >= 128, 1e12, 0)
        w = int(np.argmin(score))
        wdst[d] = w
        lo[w] += ld[d]
        hi[w] += hd[d]
        cnt[w] += 1
    # repair: move single dsts out of over-cap windows (aggregate slack is
    # ~3%, so a feasible move nearly always exists)
    for _ in range(2000):
        viol = np.where((lo > 1024) | (hi > 1024))[0]
        if len(viol) == 0:
            break
        w = int(viol[0])
        members = np.where(wdst == w)[0]
        members = members[np.argsort(-(ld[members] + hd[members]))]
        moved = False
        for d in members:
            fit = (cnt < 128) & (lo + ld[d] <= 1024) & (hi + hd[d] <= 1024)
            fit[w] = False
            if (lo[w] - ld[d] <= 1024) and (hi[w] - hd[d] <= 1024) \
                    and fit.any():
                w2 = int(np.argmin(np.where(fit, lo + hi, np.inf)))
                wdst[d] = w2
                lo[w] -= ld[d]; hi[w] -= hd[d]; cnt[w] -= 1
                lo[w2] += ld[d]; hi[w2] += hd[d]; cnt[w2] += 1
                moved = True
                break
        if not moved:
            # move the heaviest member anywhere with count room
            d = members[0]
            fit = cnt < 128
            fit[w] = False
            w2 = int(np.argmin(np.where(fit, (np.maximum(lo + ld[d] - 1024, 0)
                                              + np.maximum(hi + hd[d] - 1024, 0))
                                        * 1e6 + lo + hi, np.inf)))
            wdst[d] = w2
            lo[w] -= ld[d]; hi[w] -= hd[d]; cnt[w] -= 1
            lo[w2] += ld[d]; hi[w2] += hd[d]; cnt[w2] += 1
    assert lo.max() <= 1024 and hi.max() <= 1024 and cnt.max() <= 128, \
        (lo.max(), hi.max(), cnt.max())
    pos = np.zeros(n, np.int64)
    c2 = np.zeros(WR, np.int64)
    for d in range(n):
        pos[d] = c2[wdst[d]]
        c2[wdst[d]] += 1
    return wdst * 128 + pos


def _build_structures(edge_src, edge_dst, edge_weight):
    src = np.asarray(edge_src, np.int64)
    dst = np.asarray(edge_dst, np.int64)
    wgt = np.asarray(edge_weight, np.float32)

    # per-core region split balancing BOTH out-degree mass (bounds src-half
    # edge totals) and in-degree mass (bounds dst-window packing), size <= RS
    outdeg = np.bincount(src, minlength=N)
    indeg = np.bincount(dst, minlength=N)
    regB = np.zeros(N, bool)
    for c in range(C):
        od = outdeg[c * SHARD:(c + 1) * SHARD].astype(np.float64)
        idg = indeg[c * SHARD:(c + 1) * SHARD].astype(np.float64)
        order = np.argsort(-(od + idg), kind='stable')
        b = np.zeros(SHARD, bool)
        mo = np.zeros(2)
        mi = np.zeros(2)
        cnt2 = np.zeros(2, np.int64)
        for d in order:
            s0 = max(mo[0] + od[d], mi[0] + idg[d]) + (1e12 if cnt2[0] >= RS else 0)
            s1 = max(mo[1] + od[d], mi[1] + idg[d]) + (1e12 if cnt2[1] >= RS else 0)
            r = 0 if s0 <= s1 else 1
            b[d] = bool(r)
            mo[r] += od[d]
            mi[r] += idg[d]
            cnt2[r] += 1
        regB[c * SHARD:(c + 1) * SHARD] = b
        assert (~b).sum() <= RS and b.sum() <= RS
    e_half = regB[src].astype(np.int64)           # edge table half by src

    # per-core packing (needs per-dst degree split by src region)
    packed = np.zeros(N, np.int64)
    for c in range(C):
        mc = (dst // SHARD) == c
        ldst = dst[mc] % SHARD
        hs = e_half[mc]
        ld = np.bincount(ldst[hs == 0], minlength=SHARD)
        hd = np.bincount(ldst[hs == 1], minlength=SHARD)
        b = regB[c * SHARD:(c + 1) * SHARD]
        pk = np.empty(SHARD, np.int64)
        idxA = np.where(~b)[0]
        idxB = np.where(b)[0]
        pk[idxA] = _pack_region(ld[idxA], hd[idxA])
        pk[idxB] = RS + _pack_region(ld[idxB], hd[idxB])
        packed[c * SHARD:(c + 1) * SHARD] = pk

    # table row of a node within its region table: c*RS + (pos % RS)
    g_row = (src // SHARD) * RS + (packed[src] % RS)
    assert g_row.max() < TROWS <= 32768

    tables = []
    perms = []
    for c in range(C):
        mc = (dst // SHARD) == c
        pk = packed[c * SHARD:(c + 1) * SHARD]
        perms.append(pk)
        cs = g_row[mc]
        chh = e_half[mc]
        cd = pk[dst[mc] % SHARD]
        cw = wgt[mc] * np.float32(1.0 - ALPHA)
        order = np.lexsort((np.arange(len(cs)), cd, chh))
        cs, chh, cd, cw = cs[order], chh[order], cd[order], cw[order]
        core_t = []
        for h in (0, 1):
            sel = chh == h
            ps_, ss, ws_ = cd[sel], cs[sel], cw[sel]
            wsel = ps_ // 128
            cnt = np.bincount(wsel, minlength=W)
            assert cnt.max() <= TLW * 128
            starts = np.concatenate([[0], np.cumsum(cnt)[:-1]])
            r = np.arange(int(sel.sum())) - starts[wsel]
            tile_i = wsel * TLW + r // 128
            lane = r % 128
            idx = np.zeros((TH * 128,), np.int64)
            wp = np.zeros((128, TH), np.float32)
            dl = np.full((128, TH), -1.0, np.float32)
            idx[tile_i * 128 + lane] = ss
            wp[lane, tile_i] = ws_
            dl[lane, tile_i] = np.float32(ps_ % 128)
            core_t.append((idx.reshape(TH, 128), wp, dl))
        tables.append(core_t)
    return tables, perms


def _pack_idx(idx_t128):
    """[T,128] int ->  [128, T*8] int16 gather table (pos i=t*128+p)."""
    flat = np.asarray(idx_t128, np.int16).reshape(-1)     # i = t*128+p
    blk = flat.reshape(-1, 16).T                          # [16, T*8]
    return np.tile(blk, (8, 1)).copy()                    # [128, T*8]


# ---------------------------------------------------------------- bass build

def _build_nc():
    import concourse.bass as bass
    import concourse.bacc as bacc
    import concourse.tile as tile
    import concourse.mybir as mybir

    f32, i16, bf16 = mybir.dt.float32, mybir.dt.int16, mybir.dt.bfloat16
    Alu, Act = mybir.AluOpType, mybir.ActivationFunctionType

    nc = bacc.Bacc("TRN2", target_bir_lowering=False, debug=False,
                   num_devices=C, num_swdge_queues=4)

    xT_d = nc.dram_tensor("xT", [IN_CH, PS], bf16, kind="ExternalInput")
    idx_d = [nc.dram_tensor(f"idx{h}", [128, TH * 8], i16, kind="ExternalInput")
             for h in (0, 1)]
    wp_d = [nc.dram_tensor(f"wp{h}", [128, TH], f32, kind="ExternalInput")
            for h in (0, 1)]
    dl_d = [nc.dram_tensor(f"dl{h}", [128, TH], bf16, kind="ExternalInput")
            for h in (0, 1)]
    iota_d = nc.dram_tensor("iota", [128, 128], bf16, kind="ExternalInput")
    Win_d = nc.dram_tensor("Win", [IN_CH, HID], bf16, kind="ExternalInput")
    bin_d = nc.dram_tensor("bin", [HID, 1], f32, kind="ExternalInput")
    Wt_d = nc.dram_tensor("Wt", [HID, L * HID], f32, kind="ExternalInput")
    Wout_d = nc.dram_tensor("Wout", [HID, OUT_CH], f32, kind="ExternalInput")
    bout_d = nc.dram_tensor("bout", [OUT_CH, 1], f32, kind="ExternalInput")
    id64_d = nc.dram_tensor("id64", [64, 64], f32, kind="ExternalInput")
    id128_d = nc.dram_tensor("id128", [128, 128], f32, kind="ExternalInput")
    outT_d = nc.dram_tensor("outT", [OUT_CH, PS], f32, kind="ExternalOutput")

    with tile.TileContext(nc) as tc:
        with tc.tile_pool(name="const", bufs=1) as cp, \
             tc.tile_pool(name="msg", bufs=6) as m_pool, \
             tc.tile_pool(name="msgb", bufs=6) as mb_pool, \
             tc.tile_pool(name="sel", bufs=5) as s_pool, \
             tc.tile_pool(name="work", bufs=3) as wp_pool, \
             tc.tile_pool(name="psA", bufs=2, space="PSUM") as psA, \
             tc.tile_pool(name="psB", bufs=2, space="PSUM") as psB, \
             tc.tile_pool(name="psC", bufs=2, space="PSUM") as psC, \
             tc.tile_pool(name="dram", bufs=1, space="DRAM") as dram:

            # ---- persistent SBUF
            xT = cp.tile([IN_CH, PS], bf16)
            nc.sync.dma_start(xT[:], xT_d.ap())
            idx_sb = [cp.tile([128, TH * 8], i16, name=f"idxsb{h}") for h in (0, 1)]
            wp_sb = [cp.tile([128, TH], f32, name=f"wpsb{h}") for h in (0, 1)]
            dl_sb = [cp.tile([128, TH], bf16, name=f"dlsb{h}") for h in (0, 1)]
            for h in (0, 1):
                nc.sync.dma_start(idx_sb[h][:], idx_d[h].ap())
                nc.sync.dma_start(wp_sb[h][:], wp_d[h].ap())
                nc.sync.dma_start(dl_sb[h][:], dl_d[h].ap())
            iota = cp.tile([128, 128], bf16)
            nc.sync.dma_start(iota[:], iota_d.ap())
            Win = cp.tile([IN_CH, HID], bf16)
            nc.sync.dma_start(Win[:], Win_d.ap())
            b_in = cp.tile([HID, 1], f32)
            nc.sync.dma_start(b_in[:], bin_d.ap())
            Wt = cp.tile([HID, L * HID], f32)
            nc.sync.dma_start(Wt[:], Wt_d.ap())
            Wout = cp.tile([HID, OUT_CH], f32)
            nc.sync.dma_start(Wout[:], Wout_d.ap())
            b_out = cp.tile([OUT_CH, 1], f32)
            nc.sync.dma_start(b_out[:], bout_d.ap())
            id64 = cp.tile([64, 64], f32)
            nc.sync.dma_start(id64[:], id64_d.ap())
            id128 = cp.tile([128, 128], f32)
            nc.sync.dma_start(id128[:], id128_d.ap())

            x0s = cp.tile([128, W * HID], f32)       # 0.1*x0, node-major
            h_stage = cp.tile([128, W * HID], f32)   # new h, node-major
            outT_st = cp.tile([OUT_CH, PS], f32)

            h_sh = [[dram.tile([RS, HID], f32, name=f"h_sh{i}_{r}",
                               tag=f"hs{i}{r}") for r in (0, 1)]
                    for i in range(L)]
            h_full = [[dram.tile([TROWS, HID], f32, addr_space="Shared",
                                 name=f"h_full{i}_{r}", tag=f"hf{i}{r}")
                       for r in (0, 1)] for i in range(L)]
            h_stage_r = h_stage[:].rearrange("p (w f) -> p w f", f=HID)

            def writeback(i, w0, w1):
                # windows [w0, w1) of layer i's h -> region shard rows
                r = 0 if w0 < WR else 1
                assert (w1 <= WR) == (r == 0)
                hs_r = h_sh[i][r][:].rearrange("(w p) f -> p w f", p=128)
                nc.sync.dma_start(hs_r[:, w0 - r * WR:w1 - r * WR, :],
                                  h_stage_r[:, w0:w1, :])

            def allgather(i, r):
                nc.gpsimd.collective_compute(
                    "AllGather", Alu.bypass,
                    replica_groups=[list(range(C))],
                    ins=[h_sh[i][r].opt()], outs=[h_full[i][r].opt()])

            # ---- entry: h0 = relu(x @ Win + b), x0s = 0.1*h0 (4 windows/mm)
            for w0 in range(0, W, 4):
                nb = min(4, W - w0)
                pe = psA.tile([HID, 512], f32, name="pe", tag="psA")
                nc.tensor.matmul(pe[:, 0:nb * 128], lhsT=Win[:],
                                 rhs=xT[:, w0 * 128:(w0 + nb) * 128],
                                 start=True, stop=True)
                h0T = wp_pool.tile([HID, 512], f32, name="h0T")
                nc.scalar.activation(h0T[:, 0:nb * 128], pe[:, 0:nb * 128],
                                     Act.Relu, bias=b_in[:, 0:1])
                for k in range(nb):
                    w = w0 + k
                    pt = psB.tile([128, HID], f32, name="pt", tag="psB")
                    nc.tensor.transpose(pt[:], h0T[:, k * 128:(k + 1) * 128],
                                        id64[:])
                    nc.vector.tensor_copy(h_stage[:, w * HID:(w + 1) * HID],
                                          pt[:])
                    nc.vector.tensor_scalar_mul(
                        x0s[:, w * HID:(w + 1) * HID], pt[:], 0.1)
                if w0 + nb == 28:                 # windows 0..24 ready
                    writeback(0, 0, WR)
                    allgather(0, 0)
            writeback(0, WR, W)
            allgather(0, 1)

            qctr = 0
            # ---- layers
            for l in range(L):
                h_half = [h_full[l][0], h_full[l][1]]

                gathered = {}     # (gi, h) -> (mp, s_sb)

                def emit_gather(gi, h):
                    nonlocal qctr
                    t0 = gi * NT
                    s_sb = s_pool.tile([128, NT * 128], bf16,
                                       name=f"s{h}", tag=f"s{h}")
                    s3 = s_sb[:].rearrange("p (t j) -> p t j", j=128)
                    io_b = iota[:].unsqueeze(1).broadcast_to([128, NT, 128])
                    dl_b = dl_sb[h][:, t0:t0 + NT].unsqueeze(2) \
                        .broadcast_to([128, NT, 128])
                    nc.vector.tensor_tensor(s3, io_b, dl_b, Alu.is_equal)
                    m = m_pool.tile([128, NT * 64], f32,
                                    name=f"m{h}", tag=f"m{h}")
                    nc.gpsimd.dma_gather(
                        out_ap=m[:].rearrange("p (t f) -> p t f", f=64),
                        in_ap=h_half[h],
                        idxs_ap=idx_sb[h][:, t0 * 8:(t0 + NT) * 8],
                        num_idxs=NT * 128,
                        num_idxs_reg=NT * 128,
                        elem_size=HID,
                        single_packet=False,
                        queue_num=QPAT[qctr % 50],
                    )
                    qctr += 1
                    wb = wp_sb[h][:, t0:t0 + NT].unsqueeze(2) \
                        .broadcast_to([128, NT, 64])
                    m3 = m[:].rearrange("p (t f) -> p t f", f=64)
                    mp = mb_pool.tile([128, NT * 64], bf16,
                                      name=f"mp{h}", tag=f"mp{h}")
                    mp3 = mp[:].rearrange("p (t f) -> p t f", f=64)
                    nc.vector.tensor_tensor(mp3, m3, wb, Alu.mult)
                    gathered[(gi, h)] = (mp, s_sb)

                # ---- phase 1: all region-A gathers; accumulate the A-half
                # of each window's segment-sum into h_stage (scratch):
                # h_stage[w] = psumA + 0.1*x0.  The long A-burst keeps the
                # Pool queue busy while AG-B (doorbelled at the end of the
                # previous layer) completes.
                for gi in range(NG):
                    emit_gather(gi, 0)
                    mp, s_sb = gathered[(gi, 0)]
                    for k in range(GW):
                        w = gi * GW + k
                        ps = psA.tile([128, HID], f32, name="agg", tag="psA")
                        for t in range(TLW):
                            to = k * TLW + t
                            nc.tensor.matmul(
                                ps[:],
                                lhsT=s_sb[:, to * 128:(to + 1) * 128],
                                rhs=mp[:, to * 64:(to + 1) * 64],
                                start=(t == 0), stop=(t == TLW - 1))
                        nc.vector.tensor_tensor(
                            h_stage[:, w * HID:(w + 1) * HID], ps[:],
                            x0s[:, w * HID:(w + 1) * HID], Alu.add)
                    del gathered[(gi, 0)]

                # ---- phase 2: region-B gathers; finalize each window
                wb_mark = 0
                for gi in range(NG):
                    emit_gather(gi, 1)
                    mp, s_sb = gathered[(gi, 1)]
                    for k in range(GW):
                        w = gi * GW + k
                        ps = psA.tile([128, HID], f32, name="agg", tag="psA")
                        for t in range(TLW):
                            to = k * TLW + t
                            nc.tensor.matmul(
                                ps[:],
                                lhsT=s_sb[:, to * 128:(to + 1) * 128],
                                rhs=mp[:, to * 64:(to + 1) * 64],
                                start=(t == 0), stop=(t == TLW - 1))
                        u = wp_pool.tile([128, HID], f32, name="u")
                        nc.vector.tensor_tensor(
                            u[:], ps[:], h_stage[:, w * HID:(w + 1) * HID],
                            Alu.add)
                        pt = psB.tile([HID, 128], f32, name="ptu", tag="psB")
                        nc.tensor.transpose(pt[:], u[:], id128[:])
                        uT = wp_pool.tile([HID, 128], f32, name="uT")
                        nc.vector.tensor_copy(uT[:], pt[:])
                        if l < L - 1:
                            pd = psC.tile([128, HID], f32, name="pd", tag="psC")
                            nc.tensor.matmul(
                                pd[:], lhsT=uT[:],
                                rhs=Wt[:, l * HID:(l + 1) * HID],
                                start=True, stop=True)
                            nc.scalar.activation(
                                h_stage[:, w * HID:(w + 1) * HID], pd[:],
                                Act.Relu)
                        else:
                            pd = psC.tile([HID, 128], f32, name="pdT", tag="psC")
                            nc.tensor.matmul(
                                pd[:], lhsT=Wt[:, l * HID:(l + 1) * HID],
                                rhs=uT[:], start=True, stop=True)
                            hT = wp_pool.tile([HID, 128], f32, name="hT")
                            nc.scalar.activation(hT[:], pd[:], Act.Relu)
                            px = psA.tile([OUT_CH, 128], f32, name="px",
                                          tag="psA")
                            nc.tensor.matmul(px[:], lhsT=Wout[:], rhs=hT[:],
                                             start=True, stop=True)
                            nc.vector.tensor_scalar_add(
                                outT_st[:, w * 128:(w + 1) * 128], px[:],
                                b_out[:, 0:1])
                    del gathered[(gi, 1)]
                    if l < L - 1:
                        done_w = (gi + 1) * GW
                        if gi == AG_A_GROUP:
                            writeback(l + 1, wb_mark, WR)
                            allgather(l + 1, 0)
                            wb_mark = WR
                        elif done_w - wb_mark >= 10 and done_w <= WR:
                            writeback(l + 1, wb_mark, done_w)
                            wb_mark = done_w
                        elif wb_mark >= WR and done_w - wb_mark >= 10 \
                                and done_w < W:
                            writeback(l + 1, wb_mark, done_w)
                            wb_mark = done_w
                if l < L - 1:
                    if wb_mark < W:
                        writeback(l + 1, wb_mark, W)
                    allgather(l + 1, 1)

            nc.sync.dma_start(outT_d.ap(), outT_st[:])

    nc.compile()
    return nc


# ---------------------------------------------------------------- entry point

_CACHE = {}


def kernel(x, edge_src, edge_dst, edge_weight, W_in, b_in, W_convs, W_out,
           b_out):
    import ml_dtypes
    from concourse.bass_utils import run_bass_kernel_spmd

    x = np.asarray(x, np.float32)
    W_in = np.asarray(W_in, np.float32)
    b_in_a = np.asarray(b_in, np.float32)
    W_convs = np.asarray(W_convs, np.float32)
    W_out_a = np.asarray(W_out, np.float32)
    b_out_a = np.asarray(b_out, np.float32)

    tables, perms = _build_structures(edge_src, edge_dst, edge_weight)

    if 'nc' not in _CACHE:
        _CACHE['nc'] = _build_nc()
    nc = _CACHE['nc']

    # folded per-layer weights
    Wt = np.zeros((HID, L * HID), np.float32)
    for l in range(L):
        beta = np.float32(np.log(THETA / (l + 1) + 1.0))
        Wt[:, l * HID:(l + 1) * HID] = \
            (1 - beta) * np.eye(HID, dtype=np.float32) + beta * W_convs[l]

    in_maps = []
    for c in range(C):
        xs = np.zeros((PS, IN_CH), np.float32)
        xs[perms[c]] = x[c * SHARD:(c + 1) * SHARD]
        in_maps.append({
            "xT": np.ascontiguousarray(xs.T).astype(ml_dtypes.bfloat16),
            "idx0": _pack_idx(tables[c][0][0]),
            "idx1": _pack_idx(tables[c][1][0]),
            "wp0": tables[c][0][1],
            "wp1": tables[c][1][1],
            "dl0": tables[c][0][2].astype(ml_dtypes.bfloat16),
            "dl1": tables[c][1][2].astype(ml_dtypes.bfloat16),
            "iota": np.tile(np.arange(128, dtype=np.float32),
                            (128, 1)).astype(ml_dtypes.bfloat16),
            "Win": W_in.astype(ml_dtypes.bfloat16),
            "bin": b_in_a.reshape(HID, 1),
            "Wt": Wt,
            "Wout": W_out_a,
            "bout": b_out_a.reshape(OUT_CH, 1),
            "id64": np.eye(64, dtype=np.float32),
            "id128": np.eye(128, dtype=np.float32),
        })

    res = run_bass_kernel_spmd(nc, in_maps, core_ids=list(range(C)))
    globals()['_LAST_RESULTS'] = res

    out = np.zeros((N, OUT_CH), np.float32)
    for c in range(C):
        oT = res.results[c]["outT"]          # [40, PS]
        out[c * SHARD:(c + 1) * SHARD] = oT[:, perms[c]].T
    return out
